# revision 40
# baseline (speedup 1.0000x reference)
"""Multi-head causal self-attention on 8 trn2 NeuronCores.

Problem: x[4, 2048, 1024], 16 heads of 64 dims, causal softmax attention,
torch-Linear style projections (y = x @ W.T + b).

Sharding: core c = (batch b = c // 2, head-group g = c % 2). Each core
computes the attention output for batch b over heads [8g, 8g+8) and the
partial output projection for those heads' 512 value dims. The host sums
the two head-group partials per batch (the "all-reduce after W_O" of
tensor parallelism, done during unshard) and adds the rank-1 bias
corrections (bv @ Wo.T + bo), which commute with attention because
softmax rows sum to 1.

v2 design notes (per-engine budget from the TimelineSim cost model):
  - All matmul operands are fp16 (1 PE row/cycle, same as fp32r at
    free >= 256, but half the SBUF, 1-cycle PE transposes, and no
    gpsimd rounding ops on the critical path). fp16 end-to-end rel err
    ~1e-3 vs the 2e-2 gate.
  - PE matmul work (~200us at 2.4GHz) is the binding engine; ACT exp is
    ~160us. The query windows are processed in order w0, w1, w3, w2 so
    the exp-heaviest window (w3, ~51us of ACT) runs mid-kernel where
    projection/tail fill work still exists for the PE, and the lightest
    possible tail (w2 + its W_O) ends the kernel.
  - DMA emission order = need order: wq/wk c0, x(0), wv, x(1), wq/wk
    c1.., x(2), x(3), wq/wk c3, Wo. First scores start ~9us (gated by
    the 3MB x(0)+wq/wk c0 prefix at ~344GB/s).
  - A short burst of dummy identity transposes warms the PE p-state
    (1.2GHz -> 2.4GHz after 3us busy) during the initial DMA wait, so
    the first real projections run at full clock.
  - Scores land as s_T[k, q] pairs in 2-bank PSUM tiles so one ACT
    instruction exponentiates two k-chunks (ACT per-instruction
    overhead ~185ns). The causal mask is a multiplicative 0/1 square
    applied after exp, off the scores->exp chain. P@V' (fp16, with a
    ones column producing softmax denominators) lags one head behind
    scores/exp; 1/denom folds into the PSUM drain.
  - Windows 0-1 use head-PAIR score emission (adjacent K=64 matmuls on
    disjoint PE row halves run concurrently in the array — a real-HW
    win the cost model doesn't credit).
"""

from contextlib import ExitStack

import numpy as np

import concourse.bass as bass
import concourse.mybir as mybir
import concourse.tile as tile
from concourse import bacc
from concourse.masks import make_identity

F32 = mybir.dt.float32
F16 = mybir.dt.float16
Exp = mybir.ActivationFunctionType.Exp

D = 1024          # model dim
T = 2048          # sequence length
BATCH = 4
NH = 16           # total heads
DH = 64           # head dim
HLOC = 8          # heads per core
DSH = 512         # value dims per core (HLOC * DH)
N_CORES = 8

TC = T // 512     # 4 column tiles of 512
KC = T // 128     # 16 k chunks of 128
DC = D // 128     # 8 contraction chunks for the QKV projections

NWARM = 34        # dummy PE matmuls to ramp the p-state during DMA wait


def _build():
    nc = bacc.Bacc("TRN2", target_bir_lowering=False, debug=False,
                   num_devices=N_CORES)
    xT = nc.dram_tensor("xT", [D, T], F32, kind="ExternalInput").ap()
    wqT = nc.dram_tensor("wqT", [D, DSH], F32, kind="ExternalInput").ap()
    wkT = nc.dram_tensor("wkT", [D, DSH], F32, kind="ExternalInput").ap()
    wvT = nc.dram_tensor("wvT", [D, DSH], F32, kind="ExternalInput").ap()
    woT = nc.dram_tensor("woT", [DSH, D], F32, kind="ExternalInput").ap()
    bq = nc.dram_tensor("bq", [DSH], F32, kind="ExternalInput").ap()
    bk = nc.dram_tensor("bk", [DSH], F32, kind="ExternalInput").ap()
    y = nc.dram_tensor("y", [T, D], F32, kind="ExternalOutput").ap()

    with tile.TileContext(nc) as tc, ExitStack() as ctx:
        singles = ctx.enter_context(tc.tile_pool(name="singles", bufs=1))
        wpool = ctx.enter_context(tc.tile_pool(name="wpool", bufs=1))
        xtpool = ctx.enter_context(tc.tile_pool(name="xtpool", bufs=4))
        tmp_pool = ctx.enter_context(tc.tile_pool(name="tmp", bufs=8))
        attnp = ctx.enter_context(tc.tile_pool(name="attnp", bufs=4))
        attnTp = ctx.enter_context(tc.tile_pool(name="attnTp", bufs=3))
        exp_pool = ctx.enter_context(tc.tile_pool(name="exp", bufs=20))
        small = ctx.enter_context(tc.tile_pool(name="small", bufs=12))
        ybuf = ctx.enter_context(tc.tile_pool(name="ybuf", bufs=4))
        ps_s = ctx.enter_context(tc.tile_pool(name="ps_s", bufs=2, space="PSUM"))
        ps_pv = ctx.enter_context(tc.tile_pool(name="ps_pv", bufs=2, space="PSUM"))
        ps_fill = ctx.enter_context(tc.tile_pool(name="ps_fill", bufs=2, space="PSUM"))

        KT_t = singles.tile([128, 4, T], F16)       # [dk%128, dk//128, t]
        QT_t = singles.tile([128, 4, T], F16)       # all four windows' Q
        Vp_t = singles.tile([128, KC, HLOC, DH + 1], F16)  # [t%128, t//128, h, dv+1]
        ident_t = singles.tile([128, 128], F16)
        mask_t = singles.tile([128, 128], F16)      # 0/1 causal square
        bq_t = singles.tile([128, 4], F32)
        bk_t = singles.tile([128, 4], F32)

        def init_masks():
            """Identity + causal mask setup (gpsimd). Emitted AFTER the
            x(0) loads so these don't delay the xt converts on the Pool
            queue — first consumers are the exp-mask muls ~12us in."""
            make_identity(nc, ident_t)
            nc.vector.memset(Vp_t[:, :, :, DH:DH + 1], 1.0)
            nc.gpsimd.memset(mask_t, 1.0)
            # s_T layout [k, q]: multiplicative 0/1 causal mask for the
            # 128x128 diagonal square, applied to exp(s) AFTER the exp so
            # the mask sits off the scores->exp chain.
            nc.gpsimd.affine_select(
                out=mask_t, in_=mask_t,
                compare_op=mybir.AluOpType.is_ge,
                fill=0.0,
                base=0,
                pattern=[[1, 128]],
                channel_multiplier=-1,
            )

        def load(dst, src, eng=None):
            """DMA src (fp32 DRAM) into a staging tile, convert to fp16 on
            a compute engine (Pool by default; DVE for weight blocks so
            they skip the Pool queue behind x chunks)."""
            eng = eng or nc.gpsimd
            stage = tmp_pool.tile([128, 512], F32, tag="stage", name="stage")
            nc.sync.dma_start(out=stage, in_=src)
            eng.tensor_copy(dst, stage)

        wq_t = wpool.tile([128, 4, DC, 128], F16)
        wk_t = wpool.tile([128, 4, DC, 128], F16)
        wv_t = wpool.tile([128, DC, DSH], F16)
        wo_t = wpool.tile([128, 4, D], F16)
        wqT_r = wqT.rearrange("(d p) (c j) -> p c d j", p=128, c=4)
        wkT_r = wkT.rearrange("(d p) (c j) -> p c d j", p=128, c=4)
        wvT_r = wvT.rearrange("(d p) j -> p d j", p=128)
        woT_r = woT.rearrange("(c p) j -> p c j", p=128)
        xT_r = xT.rearrange("(d p) t -> p d t", p=128)

        xts = {}

        def ld_xt(w, d, eng=None):
            if w not in xts:
                xts[w] = xtpool.tile([128, DC, 512], F16, tag="xt", name="xt")
            load(xts[w][:, d, :], xT_r[:, d, 512 * w:512 * (w + 1)], eng=eng)

        def ld_wqk(w_t, w_r, c, hf):
            load(w_t[:, c, 4 * hf:4 * (hf + 1), :],
                 w_r[:, c, 4 * hf:4 * (hf + 1), :], eng=nc.vector)

        def ld_wv(d):
            load(wv_t[:, d, :], wvT_r[:, d, :])

        def ld_wo(c, jc):
            load(wo_t[:, c, 512 * jc:512 * (jc + 1)],
                 woT_r[:, c, 512 * jc:512 * (jc + 1)])

        # ---- projection steps ------------------------------------------
        def qkstep_fused(w, c):
            """Q and K groups for (w, c) with the d-loop interleaved so both
            track the x-window DMA chunk arrivals (used where xt(w) is still
            streaming in)."""
            psp = ps_fill.tile([128, 512], F32, tag="fill", name="psq")
            psk = ps_fill.tile([128, 512], F32, tag="fill", name="psk")
            for d in range(DC):
                nc.tensor.matmul(
                    psp, lhsT=wq_t[:, c, d, :], rhs=xts[w][:, d, :],
                    start=(d == 0), stop=(d == DC - 1))
                nc.tensor.matmul(
                    psk, lhsT=wk_t[:, c, d, :], rhs=xts[w][:, d, :],
                    start=(d == 0), stop=(d == DC - 1))
            nc.vector.tensor_scalar_add(
                QT_t[:, c, 512 * w:512 * (w + 1)], psp, bq_t[:, c:c + 1])
            nc.vector.tensor_scalar_add(
                KT_t[:, c, 512 * w:512 * (w + 1)], psk, bk_t[:, c:c + 1])

        def qstep(w, c):
            psp = ps_fill.tile([128, 512], F32, tag="fill", name="psq")
            for d in range(DC):
                nc.tensor.matmul(
                    psp, lhsT=wq_t[:, c, d, :], rhs=xts[w][:, d, :],
                    start=(d == 0), stop=(d == DC - 1))
            nc.vector.tensor_scalar_add(
                QT_t[:, c, 512 * w:512 * (w + 1)], psp, bq_t[:, c:c + 1])

        def kstep(w, c):
            psk = ps_fill.tile([128, 512], F32, tag="fill", name="psk")
            for d in range(DC):
                nc.tensor.matmul(
                    psk, lhsT=wk_t[:, c, d, :], rhs=xts[w][:, d, :],
                    start=(d == 0), stop=(d == DC - 1))
            nc.vector.tensor_scalar_add(
                KT_t[:, c, 512 * w:512 * (w + 1)], psk, bk_t[:, c:c + 1])

        def vstep(w, s):
            psv = ps_fill.tile([128, 512], F32, tag="fill", name="psv")
            for d in range(DC):
                nc.tensor.matmul(
                    psv, lhsT=xts[w][:, d, 128 * s:128 * (s + 1)],
                    rhs=wv_t[:, d, :],
                    start=(d == 0), stop=(d == DC - 1))
            nc.vector.tensor_copy(
                Vp_t[:, 4 * w + s, :, 0:DH],
                psv.rearrange("p (h v) -> p h v", h=HLOC),
            )

        # ---- attention emitters ----------------------------------------
        def emit_scores_exp(w, h, weave=()):
            """Scores+exp for head h of window w. `weave` closures (previous
            head's PV sub-chunks, fills) are emitted between score pairs so
            the PE has exp-independent work while ACT drains the pair queue
            (ps_s is only double-buffered)."""
            kmax = 4 * (w + 1)
            ch, po = h // 2, (h % 2) * 64
            weave = list(weave)
            wi = 0
            ex_buf = []
            for jp in range(kmax // 2):
                if jp >= 1 and wi < len(weave):
                    weave[wi]()
                    wi += 1
                pssb = ps_s.tile([128, 2, 512], F32, tag="pss", name="pss")
                exb = exp_pool.tile([128, 2, 512], F16, tag="ex", name="ex")
                rel0 = 2 * jp - 4 * w
                q0 = max(rel0, 0) * 128
                for sub in range(2):
                    j = 2 * jp + sub
                    nc.tensor.matmul(
                        pssb[:, sub, q0:],
                        lhsT=KT_t[po:po + 64, ch, 128 * j:128 * (j + 1)],
                        rhs=QT_t[po:po + 64, ch, 512 * w + q0:512 * (w + 1)],
                        start=True, stop=True,
                    )
                nc.scalar.activation(out=exb[:, :, q0:], in_=pssb[:, :, q0:],
                                     func=Exp, scale=0.125)
                for sub in range(2):
                    rel = 2 * jp + sub - 4 * w
                    if rel >= 0:
                        qq = rel * 128
                        nc.vector.tensor_mul(
                            exb[:, sub, qq:qq + 128],
                            exb[:, sub, qq:qq + 128], mask_t)
                ex_buf.append((exb, 0))
                ex_buf.append((exb, 1))
            while wi < len(weave):
                weave[wi]()
                wi += 1
            return ex_buf

        def emit_scores_exp_hpair(w, hp):
            """Scores + exp for the head pair (2hp, 2hp+1), k-chunks of the
            two heads interleaved so adjacent K=64 score matmuls target
            disjoint PE row groups (partition halves) and run concurrently
            in the array."""
            kmax = 4 * (w + 1)
            ch = hp
            exA, exB = [], []
            for jp in range(kmax // 2):
                pA = ps_s.tile([128, 2, 512], F32, tag="pss", name="pss")
                pB = ps_s.tile([128, 2, 512], F32, tag="pss", name="pss")
                eA = exp_pool.tile([128, 2, 512], F16, tag="ex", name="ex")
                eB = exp_pool.tile([128, 2, 512], F16, tag="ex", name="ex")
                rel0 = 2 * jp - 4 * w
                q0 = max(rel0, 0) * 128
                for sub in range(2):
                    j = 2 * jp + sub
                    nc.tensor.matmul(
                        pA[:, sub, q0:],
                        lhsT=KT_t[0:64, ch, 128 * j:128 * (j + 1)],
                        rhs=QT_t[0:64, ch, 512 * w + q0:512 * (w + 1)],
                        start=True, stop=True,
                    )
                    nc.tensor.matmul(
                        pB[:, sub, q0:],
                        lhsT=KT_t[64:128, ch, 128 * j:128 * (j + 1)],
                        rhs=QT_t[64:128, ch, 512 * w + q0:512 * (w + 1)],
                        start=True, stop=True,
                    )
                for pss, exb in ((pA, eA), (pB, eB)):
                    nc.scalar.activation(out=exb[:, :, q0:],
                                         in_=pss[:, :, q0:],
                                         func=Exp, scale=0.125)
                    for sub in range(2):
                        rel = 2 * jp + sub - 4 * w
                        if rel >= 0:
                            qq = rel * 128
                            nc.vector.tensor_mul(
                                exb[:, sub, qq:qq + 128],
                                exb[:, sub, qq:qq + 128], mask_t)
                exA += [(eA, 0), (eA, 1)]
                exB += [(eB, 0), (eB, 1)]
            return exA, exB

        attn = {}

        def get_attn(w):
            if w not in attn:
                attn[w] = attnp.tile([128, 4, DSH], F16, tag="attn",
                                     name="attn_t")
            return attn[w]

        def pv_sub(w, h, ex_buf, i):
            attn_t = get_attn(w)
            pso = ps_pv.tile([128, DH + 1], F32, tag="pso", name="pso")
            jlast = 4 * w + i
            for j in range(jlast + 1):
                exb, sub = ex_buf[j]
                nc.tensor.matmul(
                    pso,
                    lhsT=exb[:, sub, 128 * i:128 * (i + 1)],
                    rhs=Vp_t[:, j, h, :],
                    start=(j == 0), stop=(j == jlast),
                )
            rec = small.tile([128, 1], F32, tag="rec", name="rec")
            nc.vector.reciprocal(rec, pso[:, DH:DH + 1])
            nc.vector.tensor_mul(
                attn_t[:, i, DH * h:DH * (h + 1)],
                pso[:, 0:DH],
                rec.broadcast_to([128, DH]),
            )

        def pv_subs(w, h, ex_buf):
            return [lambda i=i: pv_sub(w, h, ex_buf, i) for i in range(4)]

        def emit_pv(w, h, ex_buf):
            for i in range(4):
                pv_sub(w, h, ex_buf, i)

        def tail_step(w, i, last=False):
            """Transpose + W_O + store for 128-query sub-chunk i of window
            w. The final window's psum drains go to the scalar engine
            (idle by then) instead of DVE."""
            attn_t = attn[w]
            drain = nc.scalar.copy if last else nc.vector.tensor_copy
            atT = attnTp.tile([128, 4, 128], F16, tag="attnT", name="attnT")
            pst = ps_fill.tile([128, 512], F16, tag="fill", name="pst")
            for c in range(4):
                nc.tensor.transpose(
                    pst[:, 128 * c:128 * (c + 1)],
                    attn_t[:, i, 128 * c:128 * (c + 1)], ident_t)
            drain(atT, pst.rearrange("p (c q) -> p c q", c=4))
            for jc in range(2):
                py = ps_fill.tile([128, 512], F32, tag="fill", name="py")
                for c in range(4):
                    nc.tensor.matmul(
                        py,
                        lhsT=atT[:, c, :],
                        rhs=wo_t[:, c, 512 * jc:512 * (jc + 1)],
                        start=(c == 0), stop=(c == 3),
                    )
                ysb = ybuf.tile([128, 512], F32, tag="ysb", name="ysb")
                drain(ysb, py)
                nc.sync.dma_start(
                    out=y[512 * w + 128 * i:512 * w + 128 * (i + 1),
                          512 * jc:512 * (jc + 1)],
                    in_=ysb,
                )

        # ---- static schedule -------------------------------------------
        # DMA/emission order is the program: weights/x stream in need
        # order; windows run w0, w1, w3, w2 (ACT balance); fill steps
        # (projections, loads, tails) weave between attention tasks.

        # preamble DMA order: x(0) d0, wq c0, wk c0, x(0) rest (biases
        # after d5 — first needed at the Q drain ~11us). x(0) converts go
        # to the idle DVE (327ns vs Pool's 806ns) so the first Q matmul
        # runs ~4.5us in and the fused Q/K group paces the x(0) stream.
        warmop = singles.tile([128, 128], F16)
        nc.vector.memset(warmop, 0.5)
        ld_xt(0, 0, eng=nc.vector)
        for hf in range(2):
            ld_wqk(wq_t, wqT_r, 0, hf)
        ld_xt(0, 1, eng=nc.vector)
        for hf in range(2):
            ld_wqk(wk_t, wkT_r, 0, hf)
        for d in range(2, 6):
            ld_xt(0, d)
        nc.sync.dma_start(out=bq_t, in_=bq.rearrange("(c p) -> p c", p=128))
        nc.sync.dma_start(out=bk_t, in_=bk.rearrange("(c p) -> p c", p=128))
        for d in range(6, DC):
            ld_xt(0, d)
        init_masks()

        if NWARM:
            # dummy transposes ramp the PE p-state during the DMA wait;
            # warmop is DVE-memset so the PE isn't gated on the gpsimd
            # identity/mask setup
            warm = ps_fill.tile([128, 512], F32, tag="fill", name="warm")
            for _ in range(NWARM):
                nc.tensor.matmul(warm[:, 0:128], lhsT=warmop, rhs=warmop,
                                 start=True, stop=True)

        qkstep_fused(0, 0)

        # -- window 0 (head pairs; wq/wk c1-c3 + wv stream in during the
        # window, PV deferred until V(0) is projected) -------------------
        ex0 = {}
        ex0[0], ex0[1] = emit_scores_exp_hpair(0, 0)
        for hf in range(2):
            ld_wqk(wq_t, wqT_r, 1, hf)
        for hf in range(2):
            ld_wqk(wk_t, wkT_r, 1, hf)
        qkstep_fused(0, 1)
        ex0[2], ex0[3] = emit_scores_exp_hpair(0, 1)
        for hf in range(2):
            ld_wqk(wq_t, wqT_r, 2, hf)
        for hf in range(2):
            ld_wqk(wk_t, wkT_r, 2, hf)
        for d in range(2):
            ld_wv(d)
        qkstep_fused(0, 2)
        ex0[4], ex0[5] = emit_scores_exp_hpair(0, 2)
        for hf in range(2):
            ld_wqk(wq_t, wqT_r, 3, hf)
        for hf in range(2):
            ld_wqk(wk_t, wkT_r, 3, hf)
        for d in range(2, 6):
            ld_wv(d)
        qkstep_fused(0, 3)
        ex0[6], ex0[7] = emit_scores_exp_hpair(0, 3)
        for d in range(6, DC):
            ld_wv(d)
        for s in range(4):
            vstep(0, s)
        for h in range(4):
            emit_pv(0, h, ex0[h])
        for d in range(DC):
            ld_xt(1, d)
        for h in range(4, 8):
            emit_pv(0, h, ex0[h])
        qkstep_fused(1, 0)

        # -- mid/back phases: windows 1+2 interleaved, then 3+2 ladder ---
        # Window 2's first heads run inside the w1 phase (x2 lands ~30us
        # in) and window 3 ladders against w2's last heads, so the heavy
        # exp work spreads across the whole timeline instead of bunching
        # at the end. PV of task N runs woven into task N+1's score pairs
        # (the PE stalls otherwise: ps_s is only double-buffered, so the
        # scores stream itself is exp-paced whenever ACT lags). Peak live
        # exp tiles: 16.
        ex1 = {}
        ex2 = {}
        ex3 = {}

        def sc1(hp, weave):
            ex1[2 * hp], ex1[2 * hp + 1] = emit_scores_exp_hpair(1, hp)
            for f in weave:
                f()

        # U-phase: w1 head pairs + w2 heads h0-h3
        sc1(0, [lambda: qstep(1, 1), lambda: kstep(1, 1),
                lambda: vstep(1, 0), lambda: vstep(1, 1)]
               + [lambda d=d: ld_xt(2, d) for d in range(4)])
        sc1(1, [lambda: qstep(1, 2), lambda: kstep(1, 2),
                lambda: vstep(1, 2), lambda: vstep(1, 3)]
               + [lambda d=d: ld_xt(2, d) for d in range(4, DC)]
               + pv_subs(1, 0, ex1[0]) + pv_subs(1, 1, ex1[1]))
        kstep(2, 0)
        qstep(2, 0)
        ex2[0] = emit_scores_exp(
            2, 0, [lambda: vstep(2, 0), lambda: vstep(2, 1)]
                  + pv_subs(1, 2, ex1[2]) + pv_subs(1, 3, ex1[3]))
        qstep(1, 3)
        kstep(1, 3)
        sc1(2, [lambda: vstep(2, 2), lambda: vstep(2, 3),
                lambda: kstep(2, 1), lambda: qstep(2, 1)]
               + pv_subs(2, 0, ex2[0]))
        ex2[1] = emit_scores_exp(
            2, 1, [lambda: [ld_xt(3, d) for d in range(4)]]
                  + pv_subs(1, 4, ex1[4]) + pv_subs(1, 5, ex1[5]))
        sc1(3, [lambda: [ld_xt(3, d) for d in range(4, DC)]]
               + pv_subs(2, 1, ex2[1]))
        ex2[2] = emit_scores_exp(
            2, 2, pv_subs(1, 6, ex1[6]) + pv_subs(1, 7, ex1[7]))
        ex2[3] = emit_scores_exp(
            2, 3, [lambda: kstep(3, 0), lambda: qstep(3, 0),
                   lambda: vstep(3, 0), lambda: vstep(3, 1)]
                  + pv_subs(2, 2, ex2[2]))

        # L-phase ladder: w3 heads with w2's h4-h7 at the edges — the
        # ladder ends on w2's exp-light heads so ACT drains before the
        # final tail; w0/w1 tails and the last projections fill the
        # w3-only stretch.
        ex3[0] = emit_scores_exp(
            3, 0, [lambda: vstep(3, 2), lambda: vstep(3, 3),
                   lambda: kstep(2, 2), lambda: qstep(2, 2)]
                  + pv_subs(2, 3, ex2[3]))
        ex2[4] = emit_scores_exp(
            2, 4, [lambda: ld_wo(0, 0), lambda: ld_wo(0, 1),
                   lambda: ld_wo(1, 0), lambda: ld_wo(1, 1)]
                  + pv_subs(3, 0, ex3[0]))
        ex3[1] = emit_scores_exp(
            3, 1, [lambda: qstep(3, 1), lambda: kstep(3, 1),
                   lambda: ld_wo(2, 0), lambda: ld_wo(2, 1)]
                  + pv_subs(2, 4, ex2[4]))
        ex2[5] = emit_scores_exp(
            2, 5, [lambda: ld_wo(3, 0), lambda: ld_wo(3, 1),
                   lambda: kstep(2, 3), lambda: qstep(2, 3)]
                  + pv_subs(3, 1, ex3[1]))
        ex3[2] = emit_scores_exp(
            3, 2, [lambda: tail_step(0, 0)] + pv_subs(2, 5, ex2[5]))
        ex3[3] = emit_scores_exp(
            3, 3, [lambda: tail_step(0, 1), lambda: qstep(3, 2),
                   lambda: kstep(3, 2)]
                  + pv_subs(3, 2, ex3[2]))
        ex3[4] = emit_scores_exp(
            3, 4, [lambda: tail_step(0, 2)] + pv_subs(3, 3, ex3[3]))
        ex3[5] = emit_scores_exp(
            3, 5, [lambda: tail_step(0, 3), lambda: qstep(3, 3),
                   lambda: kstep(3, 3)]
                  + pv_subs(3, 4, ex3[4]))
        ex3[6] = emit_scores_exp(
            3, 6, [lambda: tail_step(1, 0), lambda: tail_step(1, 1)]
                  + pv_subs(3, 5, ex3[5]))
        ex3[7] = emit_scores_exp(
            3, 7, [lambda: tail_step(1, 2), lambda: tail_step(1, 3)]
                  + pv_subs(3, 6, ex3[6]))
        ex2[6] = emit_scores_exp(
            2, 6, pv_subs(3, 7, ex3[7])
                  + [lambda: tail_step(3, 0), lambda: tail_step(3, 1)])
        ex2[7] = emit_scores_exp(
            2, 7, pv_subs(2, 6, ex2[6])
                  + [lambda: tail_step(3, 2), lambda: tail_step(3, 3)])
        # end: pv(2,7) sub-chunks interleaved with the final tail
        for i in range(4):
            pv_sub(2, 7, ex2[7], i)
            tail_step(2, i, last=True)
    nc.compile()
    return nc


def shard_inputs(x, Wq, bq, Wk, bk, Wv, bv, Wo, bo):
    """Returns the 8 per-core input maps."""
    in_maps = []
    for c in range(N_CORES):
        b, g = c // 2, c % 2
        sl = slice(DSH * g, DSH * (g + 1))
        in_maps.append({
            "xT": np.ascontiguousarray(x[b].T),
            "wqT": np.ascontiguousarray(Wq[sl, :].T),
            "wkT": np.ascontiguousarray(Wk[sl, :].T),
            "wvT": np.ascontiguousarray(Wv[sl, :].T),
            "woT": np.ascontiguousarray(Wo.T[sl, :]),
            "bq": np.ascontiguousarray(bq[sl]),
            "bk": np.ascontiguousarray(bk[sl]),
        })
    return in_maps


def combine_outputs(results, bv, Wo, bo):
    """Sum head-group partials per batch + rank-1 bias corrections."""
    corr = (bv @ Wo.T + bo).astype(np.float32)  # [D]; exact because softmax
    y = np.empty((BATCH, T, D), dtype=np.float32)  # rows sum to 1
    for b in range(BATCH):
        y[b] = results[2 * b]["y"] + results[2 * b + 1]["y"] + corr
    return y


def run_sharded(inputs, trace=False):
    """Build, compile, run on cores 0-7. Returns (y_full, BassKernelResults)."""
    from concourse import bass_utils

    inputs = {k: np.asarray(v, dtype=np.float32) for k, v in inputs.items()}
    nc = _build()
    in_maps = shard_inputs(
        inputs["x"], inputs["Wq"], inputs["bq"], inputs["Wk"], inputs["bk"],
        inputs["Wv"], inputs["bv"], inputs["Wo"], inputs["bo"])
    res = bass_utils.run_bass_kernel_spmd(
        nc, in_maps, list(range(N_CORES)), trace=trace)
    y = combine_outputs(res.results, inputs["bv"], inputs["Wo"], inputs["bo"])
    return y, res


def kernel(**inputs):
    y, _ = run_sharded(inputs, trace=False)
    return y


if __name__ == "__main__":
    rng = np.random.default_rng(0)
    demo = {
        "x": rng.standard_normal((BATCH, T, D), dtype=np.float32),
        "Wq": rng.standard_normal((D, D), dtype=np.float32) * 0.02,
        "bq": np.zeros(D, np.float32),
        "Wk": rng.standard_normal((D, D), dtype=np.float32) * 0.02,
        "bk": np.zeros(D, np.float32),
        "Wv": rng.standard_normal((D, D), dtype=np.float32) * 0.02,
        "bv": np.zeros(D, np.float32),
        "Wo": rng.standard_normal((D, D), dtype=np.float32) * 0.02,
        "bo": np.zeros(D, np.float32),
    }
    out = kernel(**demo)
    print(out.shape, out.dtype)


# revision 42
# speedup vs baseline: 1.0007x; 1.0007x over previous
"""Multi-head causal self-attention on 8 trn2 NeuronCores.

Problem: x[4, 2048, 1024], 16 heads of 64 dims, causal softmax attention,
torch-Linear style projections (y = x @ W.T + b).

Sharding: core c = (batch b = c // 2, head-group g = c % 2). Each core
computes the attention output for batch b over heads [8g, 8g+8) and the
partial output projection for those heads' 512 value dims. The host sums
the two head-group partials per batch (the "all-reduce after W_O" of
tensor parallelism, done during unshard) and adds the rank-1 bias
corrections (bv @ Wo.T + bo), which commute with attention because
softmax rows sum to 1.

v2 design notes (233us on the TimelineSim cost model, vs 258us for the
fp32r version; PE 88%+ occupied):
  - All matmul operands are fp16 (1 PE row/cycle, same as fp32r at
    free >= 256, but half the SBUF, 1-cycle PE transposes, and no
    mandatory rounding-producer ops). fp16 end-to-end rel err ~4e-4 vs
    the 2e-2 gate. DMA'd fp32 stages through SBUF and converts on
    Pool (x, wv, wo) or DVE (wq/wk — skips the Pool queue).
  - PE matmul work (~204us at 2.4GHz) is the binding engine and is at
    the cost-model floor (cost = out_free_size x 1 cycle/row,
    independent of contraction depth; fp8-DoubleRow would halve it but
    ~3.7% per-matmul error blows the gate). ACT exp is ~161us and is
    co-critical in the back half: the schedule's whole job is keeping
    BOTH fed.
  - Task schedule: w0 head-pairs (DMA-paced, projections interleave),
    then w1 head-pairs with w2's first heads woven in (x2 lands ~30us
    in), then a ladder alternating w3 heads against w2's remaining
    heads, ending on w2's exp-light last heads so ACT drains before
    the final tail. PV of task N is woven BETWEEN task N+1's score
    pairs: ps_s is only double-buffered, so the scores stream itself
    is exp-paced whenever ACT lags — the woven exp-independent PV/
    fill work absorbs that.
  - DMA emission order = need order: wq/wk c0 + x(0) (d0 first; first
    Q matmul ~5us), wq/wk c1-c3 streaming under w0's head-pairs, wv,
    x(1), x(2), x(3), Wo last. Biases ride between x(0) chunks.
  - NWARM dummy matmuls on a DVE-memset tile warm the PE p-state
    (1.2GHz -> 2.4GHz after 3us busy) during the initial DMA wait.
  - Scores land as s_T[k, q] pairs in 2-bank PSUM tiles so one ACT
    instruction exponentiates two k-chunks (ACT per-instruction
    overhead ~185ns). The causal mask is a multiplicative 0/1 square
    applied after exp, off the scores->exp chain. P@V' (fp16, with a
    ones column producing softmax denominators) accumulates per
    128-query sub-chunk; 1/denom folds into the PSUM drain (DVE).
  - Windows 0-1 use head-PAIR score emission (adjacent K=64 matmuls on
    disjoint PE row halves run concurrently in the array — a real-HW
    win the cost model doesn't credit).
  - W_O tails (fp16 PE transpose + matmul + fp32 store) are fill work,
    spread through the ladder; the last window's tail interleaves with
    the final PV so the end chain is drain->DMA->barrier (~4.4us).
"""

from contextlib import ExitStack

import numpy as np

import concourse.bass as bass
import concourse.mybir as mybir
import concourse.tile as tile
from concourse import bacc
from concourse.masks import make_identity

F32 = mybir.dt.float32
F16 = mybir.dt.float16
Exp = mybir.ActivationFunctionType.Exp

D = 1024          # model dim
T = 2048          # sequence length
BATCH = 4
NH = 16           # total heads
DH = 64           # head dim
HLOC = 8          # heads per core
DSH = 512         # value dims per core (HLOC * DH)
N_CORES = 8

TC = T // 512     # 4 column tiles of 512
KC = T // 128     # 16 k chunks of 128
DC = D // 128     # 8 contraction chunks for the QKV projections

NWARM = 34        # dummy PE matmuls to ramp the p-state during DMA wait


def _build():
    nc = bacc.Bacc("TRN2", target_bir_lowering=False, debug=False,
                   num_devices=N_CORES)
    xT = nc.dram_tensor("xT", [D, T], F32, kind="ExternalInput").ap()
    wqT = nc.dram_tensor("wqT", [D, DSH], F32, kind="ExternalInput").ap()
    wkT = nc.dram_tensor("wkT", [D, DSH], F32, kind="ExternalInput").ap()
    wvT = nc.dram_tensor("wvT", [D, DSH], F32, kind="ExternalInput").ap()
    woT = nc.dram_tensor("woT", [DSH, D], F32, kind="ExternalInput").ap()
    bq = nc.dram_tensor("bq", [DSH], F32, kind="ExternalInput").ap()
    bk = nc.dram_tensor("bk", [DSH], F32, kind="ExternalInput").ap()
    y = nc.dram_tensor("y", [T, D], F32, kind="ExternalOutput").ap()

    with tile.TileContext(nc) as tc, ExitStack() as ctx:
        singles = ctx.enter_context(tc.tile_pool(name="singles", bufs=1))
        wpool = ctx.enter_context(tc.tile_pool(name="wpool", bufs=1))
        xtpool = ctx.enter_context(tc.tile_pool(name="xtpool", bufs=3))
        tmp_pool = ctx.enter_context(tc.tile_pool(name="tmp", bufs=8))
        attnp = ctx.enter_context(tc.tile_pool(name="attnp", bufs=4))
        attnTp = ctx.enter_context(tc.tile_pool(name="attnTp", bufs=3))
        exp_pool = ctx.enter_context(tc.tile_pool(name="exp", bufs=18))
        small = ctx.enter_context(tc.tile_pool(name="small", bufs=12))
        ybuf = ctx.enter_context(tc.tile_pool(name="ybuf", bufs=4))
        ps_s = ctx.enter_context(tc.tile_pool(name="ps_s", bufs=2, space="PSUM"))
        ps_pv = ctx.enter_context(tc.tile_pool(name="ps_pv", bufs=2, space="PSUM"))
        ps_fill = ctx.enter_context(tc.tile_pool(name="ps_fill", bufs=2, space="PSUM"))

        KT_t = singles.tile([128, 4, T], F16)       # [dk%128, dk//128, t]
        QT_t = singles.tile([128, 4, T], F16)       # all four windows' Q
        Vp_t = singles.tile([128, KC, HLOC, DH + 1], F16)  # [t%128, t//128, h, dv+1]
        ident_t = singles.tile([128, 128], F16)
        mask_t = singles.tile([128, 128], F16)      # 0/1 causal square
        bq_t = singles.tile([128, 4], F32)
        bk_t = singles.tile([128, 4], F32)

        def init_masks():
            """Identity + causal mask setup (gpsimd). Emitted AFTER the
            x(0) loads so these don't delay the xt converts on the Pool
            queue — first consumers are the exp-mask muls ~12us in."""
            make_identity(nc, ident_t)
            nc.vector.memset(Vp_t[:, :, :, DH:DH + 1], 1.0)
            nc.gpsimd.memset(mask_t, 1.0)
            # s_T layout [k, q]: multiplicative 0/1 causal mask for the
            # 128x128 diagonal square, applied to exp(s) AFTER the exp so
            # the mask sits off the scores->exp chain.
            nc.gpsimd.affine_select(
                out=mask_t, in_=mask_t,
                compare_op=mybir.AluOpType.is_ge,
                fill=0.0,
                base=0,
                pattern=[[1, 128]],
                channel_multiplier=-1,
            )

        def load(dst, src, eng=None):
            """DMA src (fp32 DRAM) into a staging tile, convert to fp16 on
            a compute engine (Pool by default; DVE for weight blocks so
            they skip the Pool queue behind x chunks)."""
            eng = eng or nc.gpsimd
            stage = tmp_pool.tile([128, 512], F32, tag="stage", name="stage")
            nc.sync.dma_start(out=stage, in_=src)
            eng.tensor_copy(dst, stage)

        wq_t = wpool.tile([128, 4, DC, 128], F16)
        wk_t = wpool.tile([128, 4, DC, 128], F16)
        wv_t = wpool.tile([128, DC, DSH], F16)
        wo_t = wpool.tile([128, 4, D], F16)
        wqT_r = wqT.rearrange("(d p) (c j) -> p c d j", p=128, c=4)
        wkT_r = wkT.rearrange("(d p) (c j) -> p c d j", p=128, c=4)
        wvT_r = wvT.rearrange("(d p) j -> p d j", p=128)
        woT_r = woT.rearrange("(c p) j -> p c j", p=128)
        xT_r = xT.rearrange("(d p) t -> p d t", p=128)

        xts = {}

        def ld_xt(w, d, eng=None):
            if w not in xts:
                xts[w] = xtpool.tile([128, DC, 512], F16, tag="xt", name="xt")
            load(xts[w][:, d, :], xT_r[:, d, 512 * w:512 * (w + 1)], eng=eng)

        def ld_wqk(w_t, w_r, c, hf):
            load(w_t[:, c, 4 * hf:4 * (hf + 1), :],
                 w_r[:, c, 4 * hf:4 * (hf + 1), :], eng=nc.vector)

        def ld_wv(d):
            load(wv_t[:, d, :], wvT_r[:, d, :])

        def ld_wo(c, jc):
            load(wo_t[:, c, 512 * jc:512 * (jc + 1)],
                 woT_r[:, c, 512 * jc:512 * (jc + 1)])

        # ---- projection steps ------------------------------------------
        def qkstep_fused(w, c):
            """Q and K groups for (w, c) with the d-loop interleaved so both
            track the x-window DMA chunk arrivals (used where xt(w) is still
            streaming in)."""
            psp = ps_fill.tile([128, 512], F32, tag="fill", name="psq")
            psk = ps_fill.tile([128, 512], F32, tag="fill", name="psk")
            for d in range(DC):
                nc.tensor.matmul(
                    psp, lhsT=wq_t[:, c, d, :], rhs=xts[w][:, d, :],
                    start=(d == 0), stop=(d == DC - 1))
                nc.tensor.matmul(
                    psk, lhsT=wk_t[:, c, d, :], rhs=xts[w][:, d, :],
                    start=(d == 0), stop=(d == DC - 1))
            nc.vector.tensor_scalar_add(
                QT_t[:, c, 512 * w:512 * (w + 1)], psp, bq_t[:, c:c + 1])
            nc.vector.tensor_scalar_add(
                KT_t[:, c, 512 * w:512 * (w + 1)], psk, bk_t[:, c:c + 1])

        def qstep(w, c):
            psp = ps_fill.tile([128, 512], F32, tag="fill", name="psq")
            for d in range(DC):
                nc.tensor.matmul(
                    psp, lhsT=wq_t[:, c, d, :], rhs=xts[w][:, d, :],
                    start=(d == 0), stop=(d == DC - 1))
            nc.vector.tensor_scalar_add(
                QT_t[:, c, 512 * w:512 * (w + 1)], psp, bq_t[:, c:c + 1])

        def kstep(w, c):
            psk = ps_fill.tile([128, 512], F32, tag="fill", name="psk")
            for d in range(DC):
                nc.tensor.matmul(
                    psk, lhsT=wk_t[:, c, d, :], rhs=xts[w][:, d, :],
                    start=(d == 0), stop=(d == DC - 1))
            nc.vector.tensor_scalar_add(
                KT_t[:, c, 512 * w:512 * (w + 1)], psk, bk_t[:, c:c + 1])

        def vstep(w, s):
            psv = ps_fill.tile([128, 512], F32, tag="fill", name="psv")
            for d in range(DC):
                nc.tensor.matmul(
                    psv, lhsT=xts[w][:, d, 128 * s:128 * (s + 1)],
                    rhs=wv_t[:, d, :],
                    start=(d == 0), stop=(d == DC - 1))
            nc.vector.tensor_copy(
                Vp_t[:, 4 * w + s, :, 0:DH],
                psv.rearrange("p (h v) -> p h v", h=HLOC),
            )

        # ---- attention emitters ----------------------------------------
        def emit_scores_exp(w, h, weave=()):
            """Scores+exp for head h of window w. `weave` closures (previous
            head's PV sub-chunks, fills) are emitted between score pairs so
            the PE has exp-independent work while ACT drains the pair queue
            (ps_s is only double-buffered)."""
            kmax = 4 * (w + 1)
            ch, po = h // 2, (h % 2) * 64
            weave = list(weave)
            wi = 0
            ex_buf = []
            for jp in range(kmax // 2):
                if jp >= 1 and wi < len(weave):
                    weave[wi]()
                    wi += 1
                pssb = ps_s.tile([128, 2, 512], F32, tag="pss", name="pss")
                exb = exp_pool.tile([128, 2, 512], F16, tag="ex", name="ex")
                rel0 = 2 * jp - 4 * w
                q0 = max(rel0, 0) * 128
                for sub in range(2):
                    j = 2 * jp + sub
                    nc.tensor.matmul(
                        pssb[:, sub, q0:],
                        lhsT=KT_t[po:po + 64, ch, 128 * j:128 * (j + 1)],
                        rhs=QT_t[po:po + 64, ch, 512 * w + q0:512 * (w + 1)],
                        start=True, stop=True,
                    )
                nc.scalar.activation(out=exb[:, :, q0:], in_=pssb[:, :, q0:],
                                     func=Exp, scale=0.125)
                for sub in range(2):
                    rel = 2 * jp + sub - 4 * w
                    if rel >= 0:
                        qq = rel * 128
                        nc.vector.tensor_mul(
                            exb[:, sub, qq:qq + 128],
                            exb[:, sub, qq:qq + 128], mask_t)
                ex_buf.append((exb, 0))
                ex_buf.append((exb, 1))
            while wi < len(weave):
                weave[wi]()
                wi += 1
            return ex_buf

        def emit_scores_exp_hpair(w, hp):
            """Scores + exp for the head pair (2hp, 2hp+1), k-chunks of the
            two heads interleaved so adjacent K=64 score matmuls target
            disjoint PE row groups (partition halves) and run concurrently
            in the array."""
            kmax = 4 * (w + 1)
            ch = hp
            exA, exB = [], []
            for jp in range(kmax // 2):
                pA = ps_s.tile([128, 2, 512], F32, tag="pss", name="pss")
                pB = ps_s.tile([128, 2, 512], F32, tag="pss", name="pss")
                eA = exp_pool.tile([128, 2, 512], F16, tag="ex", name="ex")
                eB = exp_pool.tile([128, 2, 512], F16, tag="ex", name="ex")
                rel0 = 2 * jp - 4 * w
                q0 = max(rel0, 0) * 128
                for sub in range(2):
                    j = 2 * jp + sub
                    nc.tensor.matmul(
                        pA[:, sub, q0:],
                        lhsT=KT_t[0:64, ch, 128 * j:128 * (j + 1)],
                        rhs=QT_t[0:64, ch, 512 * w + q0:512 * (w + 1)],
                        start=True, stop=True,
                    )
                    nc.tensor.matmul(
                        pB[:, sub, q0:],
                        lhsT=KT_t[64:128, ch, 128 * j:128 * (j + 1)],
                        rhs=QT_t[64:128, ch, 512 * w + q0:512 * (w + 1)],
                        start=True, stop=True,
                    )
                for pss, exb in ((pA, eA), (pB, eB)):
                    nc.scalar.activation(out=exb[:, :, q0:],
                                         in_=pss[:, :, q0:],
                                         func=Exp, scale=0.125)
                    for sub in range(2):
                        rel = 2 * jp + sub - 4 * w
                        if rel >= 0:
                            qq = rel * 128
                            nc.vector.tensor_mul(
                                exb[:, sub, qq:qq + 128],
                                exb[:, sub, qq:qq + 128], mask_t)
                exA += [(eA, 0), (eA, 1)]
                exB += [(eB, 0), (eB, 1)]
            return exA, exB

        attn = {}

        def get_attn(w):
            if w not in attn:
                attn[w] = attnp.tile([128, 4, DSH], F16, tag="attn",
                                     name="attn_t")
            return attn[w]

        def pv_sub(w, h, ex_buf, i):
            attn_t = get_attn(w)
            pso = ps_pv.tile([128, DH + 1], F32, tag="pso", name="pso")
            jlast = 4 * w + i
            for j in range(jlast + 1):
                exb, sub = ex_buf[j]
                nc.tensor.matmul(
                    pso,
                    lhsT=exb[:, sub, 128 * i:128 * (i + 1)],
                    rhs=Vp_t[:, j, h, :],
                    start=(j == 0), stop=(j == jlast),
                )
            rec = small.tile([128, 1], F32, tag="rec", name="rec")
            nc.vector.reciprocal(rec, pso[:, DH:DH + 1])
            nc.vector.tensor_mul(
                attn_t[:, i, DH * h:DH * (h + 1)],
                pso[:, 0:DH],
                rec.broadcast_to([128, DH]),
            )

        def pv_subs(w, h, ex_buf):
            return [lambda i=i: pv_sub(w, h, ex_buf, i) for i in range(4)]

        def emit_pv(w, h, ex_buf):
            for i in range(4):
                pv_sub(w, h, ex_buf, i)

        def tail_step(w, i, last=False):
            """Transpose + W_O + store for 128-query sub-chunk i of window
            w. The final window's psum drains go to the scalar engine
            (idle by then) instead of DVE."""
            attn_t = attn[w]
            drain = nc.scalar.copy if last else nc.vector.tensor_copy
            atT = attnTp.tile([128, 4, 128], F16, tag="attnT", name="attnT")
            pst = ps_fill.tile([128, 512], F16, tag="fill", name="pst")
            for c in range(4):
                nc.tensor.transpose(
                    pst[:, 128 * c:128 * (c + 1)],
                    attn_t[:, i, 128 * c:128 * (c + 1)], ident_t)
            drain(atT, pst.rearrange("p (c q) -> p c q", c=4))
            for jc in range(2):
                py = ps_fill.tile([128, 512], F32, tag="fill", name="py")
                for c in range(4):
                    nc.tensor.matmul(
                        py,
                        lhsT=atT[:, c, :],
                        rhs=wo_t[:, c, 512 * jc:512 * (jc + 1)],
                        start=(c == 0), stop=(c == 3),
                    )
                ysb = ybuf.tile([128, 512], F32, tag="ysb", name="ysb")
                drain(ysb, py)
                nc.sync.dma_start(
                    out=y[512 * w + 128 * i:512 * w + 128 * (i + 1),
                          512 * jc:512 * (jc + 1)],
                    in_=ysb,
                )

        # ---- static schedule -------------------------------------------
        # DMA/emission order is the program: weights/x stream in need
        # order; windows run w0, w1, w3, w2 (ACT balance); fill steps
        # (projections, loads, tails) weave between attention tasks.

        # preamble DMA order: x(0) d0, wq c0, wk c0, x(0) rest (biases
        # after d5 — first needed at the Q drain ~11us). x(0) converts go
        # to the idle DVE (327ns vs Pool's 806ns) so the first Q matmul
        # runs ~4.5us in and the fused Q/K group paces the x(0) stream.
        warmop = singles.tile([128, 128], F16)
        nc.vector.memset(warmop, 0.5)
        ld_xt(0, 0, eng=nc.vector)
        for hf in range(2):
            ld_wqk(wq_t, wqT_r, 0, hf)
        ld_xt(0, 1, eng=nc.vector)
        for hf in range(2):
            ld_wqk(wk_t, wkT_r, 0, hf)
        for d in range(2, 6):
            ld_xt(0, d)
        nc.sync.dma_start(out=bq_t, in_=bq.rearrange("(c p) -> p c", p=128))
        nc.sync.dma_start(out=bk_t, in_=bk.rearrange("(c p) -> p c", p=128))
        for d in range(6, DC):
            ld_xt(0, d)
        init_masks()

        if NWARM:
            # dummy transposes ramp the PE p-state during the DMA wait;
            # warmop is DVE-memset so the PE isn't gated on the gpsimd
            # identity/mask setup
            warm = ps_fill.tile([128, 512], F32, tag="fill", name="warm")
            for _ in range(NWARM):
                nc.tensor.matmul(warm[:, 0:128], lhsT=warmop, rhs=warmop,
                                 start=True, stop=True)

        qkstep_fused(0, 0)

        # -- window 0 (head pairs; wq/wk c1-c3 + wv stream in during the
        # window, PV deferred until V(0) is projected) -------------------
        ex0 = {}
        ex0[0], ex0[1] = emit_scores_exp_hpair(0, 0)
        for hf in range(2):
            ld_wqk(wq_t, wqT_r, 1, hf)
        for hf in range(2):
            ld_wqk(wk_t, wkT_r, 1, hf)
        qkstep_fused(0, 1)
        ex0[2], ex0[3] = emit_scores_exp_hpair(0, 1)
        for hf in range(2):
            ld_wqk(wq_t, wqT_r, 2, hf)
        for hf in range(2):
            ld_wqk(wk_t, wkT_r, 2, hf)
        for d in range(2):
            ld_wv(d)
        qkstep_fused(0, 2)
        ex0[4], ex0[5] = emit_scores_exp_hpair(0, 2)
        for hf in range(2):
            ld_wqk(wq_t, wqT_r, 3, hf)
        for hf in range(2):
            ld_wqk(wk_t, wkT_r, 3, hf)
        for d in range(2, 6):
            ld_wv(d)
        qkstep_fused(0, 3)
        ex0[6], ex0[7] = emit_scores_exp_hpair(0, 3)
        for d in range(6, DC):
            ld_wv(d)
        for s in range(4):
            vstep(0, s)
        for h in range(4):
            emit_pv(0, h, ex0[h])
        for d in range(DC):
            ld_xt(1, d)
        for h in range(4, 8):
            emit_pv(0, h, ex0[h])
        qkstep_fused(1, 0)

        # -- mid/back phases: windows 1+2 interleaved, then 3+2 ladder ---
        # Window 2's first heads run inside the w1 phase (x2 lands ~30us
        # in) and window 3 ladders against w2's last heads, so the heavy
        # exp work spreads across the whole timeline instead of bunching
        # at the end. PV of task N runs woven into task N+1's score pairs
        # (the PE stalls otherwise: ps_s is only double-buffered, so the
        # scores stream itself is exp-paced whenever ACT lags). Peak live
        # exp tiles: 16.
        ex1 = {}
        ex2 = {}
        ex3 = {}

        def sc1(hp, weave):
            ex1[2 * hp], ex1[2 * hp + 1] = emit_scores_exp_hpair(1, hp)
            for f in weave:
                f()

        # U-phase: w1 head pairs + w2 heads h0-h3
        sc1(0, [lambda: qstep(1, 1), lambda: kstep(1, 1),
                lambda: vstep(1, 0), lambda: vstep(1, 1)]
               + [lambda d=d: ld_xt(2, d) for d in range(4)])
        sc1(1, [lambda: qstep(1, 2), lambda: kstep(1, 2),
                lambda: vstep(1, 2), lambda: vstep(1, 3)]
               + [lambda d=d: ld_xt(2, d) for d in range(4, DC)]
               + pv_subs(1, 0, ex1[0]) + pv_subs(1, 1, ex1[1]))
        kstep(2, 0)
        qstep(2, 0)
        ex2[0] = emit_scores_exp(
            2, 0, [lambda: vstep(2, 0), lambda: vstep(2, 1)]
                  + pv_subs(1, 2, ex1[2]) + pv_subs(1, 3, ex1[3]))
        qstep(1, 3)
        kstep(1, 3)
        sc1(2, [lambda: vstep(2, 2), lambda: vstep(2, 3),
                lambda: kstep(2, 1), lambda: qstep(2, 1)]
               + pv_subs(2, 0, ex2[0]))
        ex2[1] = emit_scores_exp(
            2, 1, [lambda: [ld_xt(3, d) for d in range(4)]]
                  + pv_subs(1, 4, ex1[4]) + pv_subs(1, 5, ex1[5]))
        sc1(3, [lambda: [ld_xt(3, d) for d in range(4, DC)]]
               + pv_subs(2, 1, ex2[1]))
        ex2[2] = emit_scores_exp(
            2, 2, pv_subs(1, 6, ex1[6]) + pv_subs(1, 7, ex1[7]))
        ex2[3] = emit_scores_exp(
            2, 3, [lambda: kstep(3, 0), lambda: qstep(3, 0),
                   lambda: vstep(3, 0), lambda: vstep(3, 1)]
                  + pv_subs(2, 2, ex2[2]))

        # L-phase ladder: w3 heads with w2's h4-h7 at the edges — the
        # ladder ends on w2's exp-light heads so ACT drains before the
        # final tail; w0/w1 tails and the last projections fill the
        # w3-only stretch.
        ex3[0] = emit_scores_exp(
            3, 0, [lambda: vstep(3, 2), lambda: vstep(3, 3),
                   lambda: kstep(2, 2), lambda: qstep(2, 2)]
                  + pv_subs(2, 3, ex2[3]))
        ex2[4] = emit_scores_exp(
            2, 4, [lambda: ld_wo(0, 0), lambda: ld_wo(0, 1),
                   lambda: ld_wo(1, 0), lambda: ld_wo(1, 1)]
                  + pv_subs(3, 0, ex3[0]))
        ex3[1] = emit_scores_exp(
            3, 1, [lambda: qstep(3, 1), lambda: kstep(3, 1),
                   lambda: ld_wo(2, 0), lambda: ld_wo(2, 1)]
                  + pv_subs(2, 4, ex2[4]))
        ex2[5] = emit_scores_exp(
            2, 5, [lambda: ld_wo(3, 0), lambda: ld_wo(3, 1),
                   lambda: kstep(2, 3), lambda: qstep(2, 3)]
                  + pv_subs(3, 1, ex3[1]))
        ex3[2] = emit_scores_exp(
            3, 2, [lambda: tail_step(0, 0)] + pv_subs(2, 5, ex2[5]))
        ex3[3] = emit_scores_exp(
            3, 3, [lambda: tail_step(0, 1), lambda: qstep(3, 2),
                   lambda: kstep(3, 2)]
                  + pv_subs(3, 2, ex3[2]))
        ex3[4] = emit_scores_exp(
            3, 4, [lambda: tail_step(0, 2)] + pv_subs(3, 3, ex3[3]))
        ex3[5] = emit_scores_exp(
            3, 5, [lambda: tail_step(0, 3), lambda: qstep(3, 3),
                   lambda: kstep(3, 3)]
                  + pv_subs(3, 4, ex3[4]))
        ex3[6] = emit_scores_exp(
            3, 6, [lambda: tail_step(1, 0), lambda: tail_step(1, 1)]
                  + pv_subs(3, 5, ex3[5]))
        ex3[7] = emit_scores_exp(
            3, 7, [lambda: tail_step(1, 2), lambda: tail_step(1, 3)]
                  + pv_subs(3, 6, ex3[6]))
        ex2[6] = emit_scores_exp(
            2, 6, pv_subs(3, 7, ex3[7])
                  + [lambda: tail_step(3, 0), lambda: tail_step(3, 1)])
        ex2[7] = emit_scores_exp(
            2, 7, pv_subs(2, 6, ex2[6])
                  + [lambda: tail_step(3, 2), lambda: tail_step(3, 3)])
        # end: pv(2,7) sub-chunks interleaved with the final tail
        for i in range(4):
            pv_sub(2, 7, ex2[7], i)
            tail_step(2, i, last=True)
    nc.compile()
    return nc


def shard_inputs(x, Wq, bq, Wk, bk, Wv, bv, Wo, bo):
    """Returns the 8 per-core input maps."""
    in_maps = []
    for c in range(N_CORES):
        b, g = c // 2, c % 2
        sl = slice(DSH * g, DSH * (g + 1))
        in_maps.append({
            "xT": np.ascontiguousarray(x[b].T),
            "wqT": np.ascontiguousarray(Wq[sl, :].T),
            "wkT": np.ascontiguousarray(Wk[sl, :].T),
            "wvT": np.ascontiguousarray(Wv[sl, :].T),
            "woT": np.ascontiguousarray(Wo.T[sl, :]),
            "bq": np.ascontiguousarray(bq[sl]),
            "bk": np.ascontiguousarray(bk[sl]),
        })
    return in_maps


def combine_outputs(results, bv, Wo, bo):
    """Sum head-group partials per batch + rank-1 bias corrections."""
    corr = (bv @ Wo.T + bo).astype(np.float32)  # [D]; exact because softmax
    y = np.empty((BATCH, T, D), dtype=np.float32)  # rows sum to 1
    for b in range(BATCH):
        y[b] = results[2 * b]["y"] + results[2 * b + 1]["y"] + corr
    return y


def run_sharded(inputs, trace=False):
    """Build, compile, run on cores 0-7. Returns (y_full, BassKernelResults)."""
    from concourse import bass_utils

    inputs = {k: np.asarray(v, dtype=np.float32) for k, v in inputs.items()}
    nc = _build()
    in_maps = shard_inputs(
        inputs["x"], inputs["Wq"], inputs["bq"], inputs["Wk"], inputs["bk"],
        inputs["Wv"], inputs["bv"], inputs["Wo"], inputs["bo"])
    res = bass_utils.run_bass_kernel_spmd(
        nc, in_maps, list(range(N_CORES)), trace=trace)
    y = combine_outputs(res.results, inputs["bv"], inputs["Wo"], inputs["bo"])
    return y, res


def kernel(**inputs):
    y, _ = run_sharded(inputs, trace=False)
    return y


if __name__ == "__main__":
    rng = np.random.default_rng(0)
    demo = {
        "x": rng.standard_normal((BATCH, T, D), dtype=np.float32),
        "Wq": rng.standard_normal((D, D), dtype=np.float32) * 0.02,
        "bq": np.zeros(D, np.float32),
        "Wk": rng.standard_normal((D, D), dtype=np.float32) * 0.02,
        "bk": np.zeros(D, np.float32),
        "Wv": rng.standard_normal((D, D), dtype=np.float32) * 0.02,
        "bv": np.zeros(D, np.float32),
        "Wo": rng.standard_normal((D, D), dtype=np.float32) * 0.02,
        "bo": np.zeros(D, np.float32),
    }
    out = kernel(**demo)
    print(out.shape, out.dtype)


# revision 43
# speedup vs baseline: 1.0049x; 1.0042x over previous
"""Multi-head causal self-attention on 8 trn2 NeuronCores.

Problem: x[4, 2048, 1024], 16 heads of 64 dims, causal softmax attention,
torch-Linear style projections (y = x @ W.T + b).

Sharding: core c = (batch b = c // 2, head-group g = c % 2). Each core
computes the attention output for batch b over heads [8g, 8g+8) and the
partial output projection for those heads' 512 value dims. The host sums
the two head-group partials per batch (the "all-reduce after W_O" of
tensor parallelism, done during unshard) and adds the rank-1 bias
corrections (bv @ Wo.T + bo), which commute with attention because
softmax rows sum to 1.

v2 design notes (233us on the TimelineSim cost model, vs 258us for the
fp32r version; PE 88%+ occupied):
  - All matmul operands are fp16 (1 PE row/cycle, same as fp32r at
    free >= 256, but half the SBUF, 1-cycle PE transposes, and no
    mandatory rounding-producer ops). fp16 end-to-end rel err ~4e-4 vs
    the 2e-2 gate. DMA'd fp32 stages through SBUF and converts on
    Pool (x, wv, wo) or DVE (wq/wk — skips the Pool queue).
  - PE matmul work (~204us at 2.4GHz) is the binding engine and is at
    the cost-model floor (cost = out_free_size x 1 cycle/row,
    independent of contraction depth; fp8-DoubleRow would halve it but
    ~3.7% per-matmul error blows the gate). ACT exp is ~161us and is
    co-critical in the back half: the schedule's whole job is keeping
    BOTH fed.
  - Task schedule: w0 head-pairs (DMA-paced, projections interleave),
    then w1 head-pairs with w2's first heads woven in (x2 lands ~30us
    in), then a ladder alternating w3 heads against w2's remaining
    heads, ending on w2's exp-light last heads so ACT drains before
    the final tail. PV of task N is woven BETWEEN task N+1's score
    pairs: ps_s is only double-buffered, so the scores stream itself
    is exp-paced whenever ACT lags — the woven exp-independent PV/
    fill work absorbs that.
  - DMA emission order = need order: wq/wk c0 + x(0) (d0 first; first
    Q matmul ~5us), wq/wk c1-c3 streaming under w0's head-pairs, wv,
    x(1), x(2), x(3), Wo last. Biases ride between x(0) chunks.
  - NWARM dummy matmuls on a DVE-memset tile warm the PE p-state
    (1.2GHz -> 2.4GHz after 3us busy) during the initial DMA wait.
  - Scores land as s_T[k, q] pairs in 2-bank PSUM tiles so one ACT
    instruction exponentiates two k-chunks (ACT per-instruction
    overhead ~185ns). The causal mask is a multiplicative 0/1 square
    applied after exp, off the scores->exp chain. P@V' (fp16, with a
    ones column producing softmax denominators) accumulates per
    128-query sub-chunk; 1/denom folds into the PSUM drain (DVE).
  - Windows 0-1 use head-PAIR score emission (adjacent K=64 matmuls on
    disjoint PE row halves run concurrently in the array — a real-HW
    win the cost model doesn't credit).
  - W_O tails (fp16 PE transpose + matmul + fp32 store) are fill work,
    spread through the ladder; the last window's tail interleaves with
    the final PV so the end chain is drain->DMA->barrier (~4.4us).
"""

from contextlib import ExitStack

import numpy as np

import concourse.bass as bass
import concourse.mybir as mybir
import concourse.tile as tile
from concourse import bacc
from concourse.masks import make_identity

F32 = mybir.dt.float32
F16 = mybir.dt.float16
Exp = mybir.ActivationFunctionType.Exp

D = 1024          # model dim
T = 2048          # sequence length
BATCH = 4
NH = 16           # total heads
DH = 64           # head dim
HLOC = 8          # heads per core
DSH = 512         # value dims per core (HLOC * DH)
N_CORES = 8

TC = T // 512     # 4 column tiles of 512
KC = T // 128     # 16 k chunks of 128
DC = D // 128     # 8 contraction chunks for the QKV projections

NWARM = 34        # dummy PE matmuls to ramp the p-state during DMA wait


def _build():
    nc = bacc.Bacc("TRN2", target_bir_lowering=False, debug=False,
                   num_devices=N_CORES)
    xT = nc.dram_tensor("xT", [D, T], F32, kind="ExternalInput").ap()
    wqT = nc.dram_tensor("wqT", [D, DSH], F32, kind="ExternalInput").ap()
    wkT = nc.dram_tensor("wkT", [D, DSH], F32, kind="ExternalInput").ap()
    wvT = nc.dram_tensor("wvT", [D, DSH], F32, kind="ExternalInput").ap()
    woT = nc.dram_tensor("woT", [DSH, D], F32, kind="ExternalInput").ap()
    bq = nc.dram_tensor("bq", [DSH], F32, kind="ExternalInput").ap()
    bk = nc.dram_tensor("bk", [DSH], F32, kind="ExternalInput").ap()
    y = nc.dram_tensor("y", [T, D], F32, kind="ExternalOutput").ap()

    with tile.TileContext(nc) as tc, ExitStack() as ctx:
        singles = ctx.enter_context(tc.tile_pool(name="singles", bufs=1))
        wpool = ctx.enter_context(tc.tile_pool(name="wpool", bufs=1))
        xtpool = ctx.enter_context(tc.tile_pool(name="xtpool", bufs=3))
        tmp_pool = ctx.enter_context(tc.tile_pool(name="tmp", bufs=8))
        attnp = ctx.enter_context(tc.tile_pool(name="attnp", bufs=4))
        attnTp = ctx.enter_context(tc.tile_pool(name="attnTp", bufs=3))
        exp_pool = ctx.enter_context(tc.tile_pool(name="exp", bufs=18))
        small = ctx.enter_context(tc.tile_pool(name="small", bufs=12))
        ybuf = ctx.enter_context(tc.tile_pool(name="ybuf", bufs=4))
        ps_s = ctx.enter_context(tc.tile_pool(name="ps_s", bufs=2, space="PSUM"))
        ps_pv = ctx.enter_context(tc.tile_pool(name="ps_pv", bufs=2, space="PSUM"))
        ps_fill = ctx.enter_context(tc.tile_pool(name="ps_fill", bufs=2, space="PSUM"))

        KT_t = singles.tile([128, 4, T], F16)       # [dk%128, dk//128, t]
        QT_t = singles.tile([128, 4, T], F16)       # all four windows' Q
        Vp_t = singles.tile([128, KC, HLOC, DH + 1], F16)  # [t%128, t//128, h, dv+1]
        ident_t = singles.tile([128, 128], F16)
        mask_t = singles.tile([128, 128], F16)      # 0/1 causal square
        bq_t = singles.tile([128, 4], F32)
        bk_t = singles.tile([128, 4], F32)

        def init_masks():
            """Identity + causal mask setup (gpsimd). Emitted AFTER the
            x(0) loads so these don't delay the xt converts on the Pool
            queue — first consumers are the exp-mask muls ~12us in."""
            make_identity(nc, ident_t)
            nc.vector.memset(Vp_t[:, :, :, DH:DH + 1], 1.0)
            nc.gpsimd.memset(mask_t, 1.0)
            # s_T layout [k, q]: multiplicative 0/1 causal mask for the
            # 128x128 diagonal square, applied to exp(s) AFTER the exp so
            # the mask sits off the scores->exp chain.
            nc.gpsimd.affine_select(
                out=mask_t, in_=mask_t,
                compare_op=mybir.AluOpType.is_ge,
                fill=0.0,
                base=0,
                pattern=[[1, 128]],
                channel_multiplier=-1,
            )

        def load(dst, src, eng=None):
            """DMA src (fp32 DRAM) into a staging tile, convert to fp16 on
            a compute engine (Pool by default; DVE for weight blocks so
            they skip the Pool queue behind x chunks)."""
            eng = eng or nc.gpsimd
            stage = tmp_pool.tile([128, 512], F32, tag="stage", name="stage")
            nc.sync.dma_start(out=stage, in_=src)
            eng.tensor_copy(dst, stage)

        wq_t = wpool.tile([128, 4, DC, 128], F16)
        wk_t = wpool.tile([128, 4, DC, 128], F16)
        wv_t = wpool.tile([128, DC, DSH], F16)
        wo_t = wpool.tile([128, 4, D], F16)
        wqT_r = wqT.rearrange("(d p) (c j) -> p c d j", p=128, c=4)
        wkT_r = wkT.rearrange("(d p) (c j) -> p c d j", p=128, c=4)
        wvT_r = wvT.rearrange("(d p) j -> p d j", p=128)
        woT_r = woT.rearrange("(c p) j -> p c j", p=128)
        xT_r = xT.rearrange("(d p) t -> p d t", p=128)

        xts = {}

        def ld_xt(w, d, eng=None):
            if w not in xts:
                xts[w] = xtpool.tile([128, DC, 512], F16, tag="xt", name="xt")
            load(xts[w][:, d, :], xT_r[:, d, 512 * w:512 * (w + 1)], eng=eng)

        def ld_wqk(w_t, w_r, c, hf):
            load(w_t[:, c, 4 * hf:4 * (hf + 1), :],
                 w_r[:, c, 4 * hf:4 * (hf + 1), :], eng=nc.vector)

        def ld_wv(d):
            load(wv_t[:, d, :], wvT_r[:, d, :])

        def ld_wo(c, jc):
            load(wo_t[:, c, 512 * jc:512 * (jc + 1)],
                 woT_r[:, c, 512 * jc:512 * (jc + 1)])

        # ---- projection steps ------------------------------------------
        def qkstep_fused(w, c):
            """Q and K groups for (w, c) with the d-loop interleaved so both
            track the x-window DMA chunk arrivals (used where xt(w) is still
            streaming in)."""
            psp = ps_fill.tile([128, 512], F32, tag="fill", name="psq")
            psk = ps_fill.tile([128, 512], F32, tag="fill", name="psk")
            for d in range(DC):
                nc.tensor.matmul(
                    psp, lhsT=wq_t[:, c, d, :], rhs=xts[w][:, d, :],
                    start=(d == 0), stop=(d == DC - 1))
                nc.tensor.matmul(
                    psk, lhsT=wk_t[:, c, d, :], rhs=xts[w][:, d, :],
                    start=(d == 0), stop=(d == DC - 1))
            nc.vector.tensor_scalar_add(
                QT_t[:, c, 512 * w:512 * (w + 1)], psp, bq_t[:, c:c + 1])
            nc.vector.tensor_scalar_add(
                KT_t[:, c, 512 * w:512 * (w + 1)], psk, bk_t[:, c:c + 1])

        def qstep(w, c):
            psp = ps_fill.tile([128, 512], F32, tag="fill", name="psq")
            for d in range(DC):
                nc.tensor.matmul(
                    psp, lhsT=wq_t[:, c, d, :], rhs=xts[w][:, d, :],
                    start=(d == 0), stop=(d == DC - 1))
            nc.vector.tensor_scalar_add(
                QT_t[:, c, 512 * w:512 * (w + 1)], psp, bq_t[:, c:c + 1])

        def kstep(w, c):
            psk = ps_fill.tile([128, 512], F32, tag="fill", name="psk")
            for d in range(DC):
                nc.tensor.matmul(
                    psk, lhsT=wk_t[:, c, d, :], rhs=xts[w][:, d, :],
                    start=(d == 0), stop=(d == DC - 1))
            nc.vector.tensor_scalar_add(
                KT_t[:, c, 512 * w:512 * (w + 1)], psk, bk_t[:, c:c + 1])

        def vstep(w, s):
            psv = ps_fill.tile([128, 512], F32, tag="fill", name="psv")
            for d in range(DC):
                nc.tensor.matmul(
                    psv, lhsT=xts[w][:, d, 128 * s:128 * (s + 1)],
                    rhs=wv_t[:, d, :],
                    start=(d == 0), stop=(d == DC - 1))
            nc.vector.tensor_copy(
                Vp_t[:, 4 * w + s, :, 0:DH],
                psv.rearrange("p (h v) -> p h v", h=HLOC),
            )

        # ---- attention emitters ----------------------------------------
        def emit_scores_exp(w, h, weave=()):
            """Scores+exp for head h of window w. `weave` closures (previous
            head's PV sub-chunks, fills) are emitted between score pairs so
            the PE has exp-independent work while ACT drains the pair queue
            (ps_s is only double-buffered)."""
            kmax = 4 * (w + 1)
            ch, po = h // 2, (h % 2) * 64
            weave = list(weave)
            wi = 0
            ex_buf = []
            for jp in range(kmax // 2):
                if jp >= 1 and wi < len(weave):
                    weave[wi]()
                    wi += 1
                pssb = ps_s.tile([128, 2, 512], F32, tag="pss", name="pss")
                exb = exp_pool.tile([128, 2, 512], F16, tag="ex", name="ex")
                rel0 = 2 * jp - 4 * w
                q0 = max(rel0, 0) * 128
                for sub in range(2):
                    j = 2 * jp + sub
                    nc.tensor.matmul(
                        pssb[:, sub, q0:],
                        lhsT=KT_t[po:po + 64, ch, 128 * j:128 * (j + 1)],
                        rhs=QT_t[po:po + 64, ch, 512 * w + q0:512 * (w + 1)],
                        start=True, stop=True,
                    )
                nc.scalar.activation(out=exb[:, :, q0:], in_=pssb[:, :, q0:],
                                     func=Exp, scale=0.125)
                for sub in range(2):
                    rel = 2 * jp + sub - 4 * w
                    if rel >= 0:
                        qq = rel * 128
                        nc.vector.tensor_mul(
                            exb[:, sub, qq:qq + 128],
                            exb[:, sub, qq:qq + 128], mask_t)
                ex_buf.append((exb, 0))
                ex_buf.append((exb, 1))
            while wi < len(weave):
                weave[wi]()
                wi += 1
            return ex_buf

        def emit_scores_exp_hpair(w, hp):
            """Scores + exp for the head pair (2hp, 2hp+1), k-chunks of the
            two heads interleaved so adjacent K=64 score matmuls target
            disjoint PE row groups (partition halves) and run concurrently
            in the array."""
            kmax = 4 * (w + 1)
            ch = hp
            exA, exB = [], []
            for jp in range(kmax // 2):
                pA = ps_s.tile([128, 2, 512], F32, tag="pss", name="pss")
                pB = ps_s.tile([128, 2, 512], F32, tag="pss", name="pss")
                eA = exp_pool.tile([128, 2, 512], F16, tag="ex", name="ex")
                eB = exp_pool.tile([128, 2, 512], F16, tag="ex", name="ex")
                rel0 = 2 * jp - 4 * w
                q0 = max(rel0, 0) * 128
                for sub in range(2):
                    j = 2 * jp + sub
                    nc.tensor.matmul(
                        pA[:, sub, q0:],
                        lhsT=KT_t[0:64, ch, 128 * j:128 * (j + 1)],
                        rhs=QT_t[0:64, ch, 512 * w + q0:512 * (w + 1)],
                        start=True, stop=True,
                    )
                    nc.tensor.matmul(
                        pB[:, sub, q0:],
                        lhsT=KT_t[64:128, ch, 128 * j:128 * (j + 1)],
                        rhs=QT_t[64:128, ch, 512 * w + q0:512 * (w + 1)],
                        start=True, stop=True,
                    )
                for pss, exb in ((pA, eA), (pB, eB)):
                    nc.scalar.activation(out=exb[:, :, q0:],
                                         in_=pss[:, :, q0:],
                                         func=Exp, scale=0.125)
                    for sub in range(2):
                        rel = 2 * jp + sub - 4 * w
                        if rel >= 0:
                            qq = rel * 128
                            nc.vector.tensor_mul(
                                exb[:, sub, qq:qq + 128],
                                exb[:, sub, qq:qq + 128], mask_t)
                exA += [(eA, 0), (eA, 1)]
                exB += [(eB, 0), (eB, 1)]
            return exA, exB

        attn = {}

        def get_attn(w):
            if w not in attn:
                attn[w] = attnp.tile([128, 4, DSH], F16, tag="attn",
                                     name="attn_t")
            return attn[w]

        def pv_sub(w, h, ex_buf, i):
            attn_t = get_attn(w)
            pso = ps_pv.tile([128, DH + 1], F32, tag="pso", name="pso")
            jlast = 4 * w + i
            for j in range(jlast + 1):
                exb, sub = ex_buf[j]
                nc.tensor.matmul(
                    pso,
                    lhsT=exb[:, sub, 128 * i:128 * (i + 1)],
                    rhs=Vp_t[:, j, h, :],
                    start=(j == 0), stop=(j == jlast),
                )
            rec = small.tile([128, 1], F32, tag="rec", name="rec")
            nc.vector.reciprocal(rec, pso[:, DH:DH + 1])
            nc.vector.tensor_mul(
                attn_t[:, i, DH * h:DH * (h + 1)],
                pso[:, 0:DH],
                rec.broadcast_to([128, DH]),
            )

        def pv_subs(w, h, ex_buf):
            return [lambda i=i: pv_sub(w, h, ex_buf, i) for i in range(4)]

        def emit_pv(w, h, ex_buf):
            for i in range(4):
                pv_sub(w, h, ex_buf, i)

        def tail_step(w, i, last=False):
            """Transpose + W_O + store for 128-query sub-chunk i of window
            w. The final window's psum drains go to the scalar engine
            (idle by then) instead of DVE."""
            attn_t = attn[w]
            drain = nc.vector.tensor_copy
            atT = attnTp.tile([128, 4, 128], F16, tag="attnT", name="attnT")
            pst = ps_fill.tile([128, 512], F16, tag="fill", name="pst")
            for c in range(4):
                nc.tensor.transpose(
                    pst[:, 128 * c:128 * (c + 1)],
                    attn_t[:, i, 128 * c:128 * (c + 1)], ident_t)
            drain(atT, pst.rearrange("p (c q) -> p c q", c=4))
            for jc in range(2):
                py = ps_fill.tile([128, 512], F32, tag="fill", name="py")
                for c in range(4):
                    nc.tensor.matmul(
                        py,
                        lhsT=atT[:, c, :],
                        rhs=wo_t[:, c, 512 * jc:512 * (jc + 1)],
                        start=(c == 0), stop=(c == 3),
                    )
                ysb = ybuf.tile([128, 512], F32, tag="ysb", name="ysb")
                drain(ysb, py)
                nc.sync.dma_start(
                    out=y[512 * w + 128 * i:512 * w + 128 * (i + 1),
                          512 * jc:512 * (jc + 1)],
                    in_=ysb,
                )

        # ---- static schedule -------------------------------------------
        # DMA/emission order is the program: weights/x stream in need
        # order; windows run w0, w1, w3, w2 (ACT balance); fill steps
        # (projections, loads, tails) weave between attention tasks.

        # preamble DMA order: x(0) d0, wq c0, wk c0, x(0) rest (biases
        # after d5 — first needed at the Q drain ~11us). x(0) converts go
        # to the idle DVE (327ns vs Pool's 806ns) so the first Q matmul
        # runs ~4.5us in and the fused Q/K group paces the x(0) stream.
        warmop = singles.tile([128, 128], F16)
        nc.vector.memset(warmop, 0.5)
        ld_xt(0, 0, eng=nc.vector)
        for hf in range(2):
            ld_wqk(wq_t, wqT_r, 0, hf)
        ld_xt(0, 1, eng=nc.vector)
        for hf in range(2):
            ld_wqk(wk_t, wkT_r, 0, hf)
        for d in range(2, 6):
            ld_xt(0, d)
        nc.sync.dma_start(out=bq_t, in_=bq.rearrange("(c p) -> p c", p=128))
        nc.sync.dma_start(out=bk_t, in_=bk.rearrange("(c p) -> p c", p=128))
        for d in range(6, DC):
            ld_xt(0, d)
        init_masks()

        if NWARM:
            # dummy transposes ramp the PE p-state during the DMA wait;
            # warmop is DVE-memset so the PE isn't gated on the gpsimd
            # identity/mask setup
            warm = ps_fill.tile([128, 512], F32, tag="fill", name="warm")
            for _ in range(NWARM):
                nc.tensor.matmul(warm[:, 0:128], lhsT=warmop, rhs=warmop,
                                 start=True, stop=True)

        qkstep_fused(0, 0)

        # -- window 0 (head pairs; wq/wk c1-c3 + wv stream in during the
        # window, PV deferred until V(0) is projected) -------------------
        ex0 = {}
        ex0[0], ex0[1] = emit_scores_exp_hpair(0, 0)
        for hf in range(2):
            ld_wqk(wq_t, wqT_r, 1, hf)
        for hf in range(2):
            ld_wqk(wk_t, wkT_r, 1, hf)
        qkstep_fused(0, 1)
        ex0[2], ex0[3] = emit_scores_exp_hpair(0, 1)
        for hf in range(2):
            ld_wqk(wq_t, wqT_r, 2, hf)
        for hf in range(2):
            ld_wqk(wk_t, wkT_r, 2, hf)
        for d in range(2):
            ld_wv(d)
        qkstep_fused(0, 2)
        ex0[4], ex0[5] = emit_scores_exp_hpair(0, 2)
        for hf in range(2):
            ld_wqk(wq_t, wqT_r, 3, hf)
        for hf in range(2):
            ld_wqk(wk_t, wkT_r, 3, hf)
        for d in range(2, 6):
            ld_wv(d)
        qkstep_fused(0, 3)
        ex0[6], ex0[7] = emit_scores_exp_hpair(0, 3)
        for d in range(6, DC):
            ld_wv(d)
        for s in range(4):
            vstep(0, s)
        for h in range(4):
            emit_pv(0, h, ex0[h])
        for d in range(DC):
            ld_xt(1, d)
        for h in range(4, 8):
            emit_pv(0, h, ex0[h])
        qkstep_fused(1, 0)

        # -- mid/back phases: windows 1+2 interleaved, then 3+2 ladder ---
        # Window 2's first heads run inside the w1 phase (x2 lands ~30us
        # in) and window 3 ladders against w2's last heads, so the heavy
        # exp work spreads across the whole timeline instead of bunching
        # at the end. PV of task N runs woven into task N+1's score pairs
        # (the PE stalls otherwise: ps_s is only double-buffered, so the
        # scores stream itself is exp-paced whenever ACT lags). Peak live
        # exp tiles: 16.
        ex1 = {}
        ex2 = {}
        ex3 = {}

        def sc1(hp, weave):
            ex1[2 * hp], ex1[2 * hp + 1] = emit_scores_exp_hpair(1, hp)
            for f in weave:
                f()

        # U-phase: w1 head pairs + w2 heads h0-h3
        sc1(0, [lambda: qstep(1, 1), lambda: kstep(1, 1),
                lambda: vstep(1, 0), lambda: vstep(1, 1)]
               + [lambda d=d: ld_xt(2, d) for d in range(4)])
        sc1(1, [lambda: qstep(1, 2), lambda: kstep(1, 2),
                lambda: vstep(1, 2), lambda: vstep(1, 3)]
               + [lambda d=d: ld_xt(2, d) for d in range(4, DC)]
               + pv_subs(1, 0, ex1[0]) + pv_subs(1, 1, ex1[1]))
        kstep(2, 0)
        qstep(2, 0)
        ex2[0] = emit_scores_exp(
            2, 0, [lambda: vstep(2, 0), lambda: vstep(2, 1)]
                  + pv_subs(1, 2, ex1[2]) + pv_subs(1, 3, ex1[3]))
        qstep(1, 3)
        kstep(1, 3)
        sc1(2, [lambda: vstep(2, 2), lambda: vstep(2, 3),
                lambda: kstep(2, 1), lambda: qstep(2, 1)]
               + pv_subs(2, 0, ex2[0]))
        ex2[1] = emit_scores_exp(
            2, 1, [lambda: [ld_xt(3, d) for d in range(4)]]
                  + pv_subs(1, 4, ex1[4]) + pv_subs(1, 5, ex1[5]))
        sc1(3, [lambda: [ld_xt(3, d) for d in range(4, DC)]]
               + pv_subs(2, 1, ex2[1]))
        ex2[2] = emit_scores_exp(
            2, 2, pv_subs(1, 6, ex1[6]) + pv_subs(1, 7, ex1[7]))
        ex2[3] = emit_scores_exp(
            2, 3, [lambda: kstep(3, 0), lambda: qstep(3, 0),
                   lambda: vstep(3, 0), lambda: vstep(3, 1)]
                  + pv_subs(2, 2, ex2[2]))

        # L-phase ladder: w3 heads with w2's h4-h7 at the edges — the
        # ladder ends on w2's exp-light heads so ACT drains before the
        # final tail; w0/w1 tails and the last projections fill the
        # w3-only stretch.
        ex3[0] = emit_scores_exp(
            3, 0, [lambda: vstep(3, 2), lambda: vstep(3, 3),
                   lambda: kstep(2, 2), lambda: qstep(2, 2)]
                  + pv_subs(2, 3, ex2[3]))
        ex2[4] = emit_scores_exp(
            2, 4, [lambda: ld_wo(0, 0), lambda: ld_wo(0, 1),
                   lambda: ld_wo(1, 0), lambda: ld_wo(1, 1)]
                  + pv_subs(3, 0, ex3[0]))
        ex3[1] = emit_scores_exp(
            3, 1, [lambda: qstep(3, 1), lambda: kstep(3, 1),
                   lambda: ld_wo(2, 0), lambda: ld_wo(2, 1)]
                  + pv_subs(2, 4, ex2[4]))
        ex2[5] = emit_scores_exp(
            2, 5, [lambda: ld_wo(3, 0), lambda: ld_wo(3, 1),
                   lambda: kstep(2, 3), lambda: qstep(2, 3)]
                  + pv_subs(3, 1, ex3[1]))
        ex3[2] = emit_scores_exp(
            3, 2, [lambda: tail_step(0, 0)] + pv_subs(2, 5, ex2[5]))
        ex3[3] = emit_scores_exp(
            3, 3, [lambda: tail_step(0, 1), lambda: qstep(3, 2),
                   lambda: kstep(3, 2)]
                  + pv_subs(3, 2, ex3[2]))
        ex3[4] = emit_scores_exp(
            3, 4, [lambda: tail_step(0, 2)] + pv_subs(3, 3, ex3[3]))
        ex3[5] = emit_scores_exp(
            3, 5, [lambda: tail_step(0, 3), lambda: qstep(3, 3),
                   lambda: kstep(3, 3)]
                  + pv_subs(3, 4, ex3[4]))
        ex3[6] = emit_scores_exp(
            3, 6, [lambda: tail_step(1, 0), lambda: tail_step(1, 1)]
                  + pv_subs(3, 5, ex3[5]))
        ex3[7] = emit_scores_exp(
            3, 7, [lambda: tail_step(1, 2), lambda: tail_step(1, 3)]
                  + pv_subs(3, 6, ex3[6]))
        ex2[6] = emit_scores_exp(
            2, 6, pv_subs(3, 7, ex3[7])
                  + [lambda: tail_step(3, 0), lambda: tail_step(3, 1)])
        ex2[7] = emit_scores_exp(
            2, 7, pv_subs(2, 6, ex2[6])
                  + [lambda: tail_step(3, 2), lambda: tail_step(3, 3)])
        # end: pv(2,7) sub-chunks interleaved with the final tail
        for i in range(4):
            pv_sub(2, 7, ex2[7], i)
            tail_step(2, i, last=True)
    nc.compile()
    return nc


def shard_inputs(x, Wq, bq, Wk, bk, Wv, bv, Wo, bo):
    """Returns the 8 per-core input maps."""
    in_maps = []
    for c in range(N_CORES):
        b, g = c // 2, c % 2
        sl = slice(DSH * g, DSH * (g + 1))
        in_maps.append({
            "xT": np.ascontiguousarray(x[b].T),
            "wqT": np.ascontiguousarray(Wq[sl, :].T),
            "wkT": np.ascontiguousarray(Wk[sl, :].T),
            "wvT": np.ascontiguousarray(Wv[sl, :].T),
            "woT": np.ascontiguousarray(Wo.T[sl, :]),
            "bq": np.ascontiguousarray(bq[sl]),
            "bk": np.ascontiguousarray(bk[sl]),
        })
    return in_maps


def combine_outputs(results, bv, Wo, bo):
    """Sum head-group partials per batch + rank-1 bias corrections."""
    corr = (bv @ Wo.T + bo).astype(np.float32)  # [D]; exact because softmax
    y = np.empty((BATCH, T, D), dtype=np.float32)  # rows sum to 1
    for b in range(BATCH):
        y[b] = results[2 * b]["y"] + results[2 * b + 1]["y"] + corr
    return y


def run_sharded(inputs, trace=False):
    """Build, compile, run on cores 0-7. Returns (y_full, BassKernelResults)."""
    from concourse import bass_utils

    inputs = {k: np.asarray(v, dtype=np.float32) for k, v in inputs.items()}
    nc = _build()
    in_maps = shard_inputs(
        inputs["x"], inputs["Wq"], inputs["bq"], inputs["Wk"], inputs["bk"],
        inputs["Wv"], inputs["bv"], inputs["Wo"], inputs["bo"])
    res = bass_utils.run_bass_kernel_spmd(
        nc, in_maps, list(range(N_CORES)), trace=trace)
    y = combine_outputs(res.results, inputs["bv"], inputs["Wo"], inputs["bo"])
    return y, res


def kernel(**inputs):
    y, _ = run_sharded(inputs, trace=False)
    return y


if __name__ == "__main__":
    rng = np.random.default_rng(0)
    demo = {
        "x": rng.standard_normal((BATCH, T, D), dtype=np.float32),
        "Wq": rng.standard_normal((D, D), dtype=np.float32) * 0.02,
        "bq": np.zeros(D, np.float32),
        "Wk": rng.standard_normal((D, D), dtype=np.float32) * 0.02,
        "bk": np.zeros(D, np.float32),
        "Wv": rng.standard_normal((D, D), dtype=np.float32) * 0.02,
        "bv": np.zeros(D, np.float32),
        "Wo": rng.standard_normal((D, D), dtype=np.float32) * 0.02,
        "bo": np.zeros(D, np.float32),
    }
    out = kernel(**demo)
    print(out.shape, out.dtype)
